# revision 14
# baseline (speedup 1.0000x reference)
"""CrossMerge kernel for trn2.

Math (per batch element):
    means_i = mean over C of g_i              (4, H, W)
    logits  = w_proj @ means + b_proj         (4, H, W)
    w       = softmax(logits, axis=0)         (4, H, W)
    out     = sum_i g_i * w_i                 (C, H, W)

Sharding: data-parallel over batch B=8 across 8 cores; weights replicated;
no cross-device communication.

Per-core layout: the 4 grids are host-stacked into gall (4, 256, 9216);
C=256 is split into 2 partition-chunks of 128.  Spatial axis tiled into
9 DMA tiles of 1024 cols (one 4 MB load + one 1 MB store each), each
split into 2 compute slices of 512 cols (fp32 PSUM bank width).

Per 512-col slice j the work is:
  PE  : 8 mm  logits L(4,512)  += ws_i(128,4)^T g_ic(128,512)   [fp32r]
        1 mm  S1(1,512) = ones4^T E         (softmax denominator)
        1 mm  R4(4,512) = broadcast R       (K=1)
        4 mm  Wb_i(128,512) = row-select broadcast of W4[i]
        8 mm  F_c(128,512) += I^T p_ic      (PSUM accumulation)
  ACT : E = exp(L + b)  [single table set];  3x copy Wb PSUM->SBUF
  DVE : R = reciprocal_approx_fast(S1);  W4 = E * R4;
        products for DVE grids; 2x F PSUM->out-tile copies
  POOL: products for remaining grids (SBUF operands only)

The d-loop is software-pipelined: products of iter d are issued in pass 1
of d, and the PE accumulation + output copy/store (pass 2) are emitted
before pass 1 of iter d+1, so the PE never sits idle waiting on the
product chain (keeps the HAM clock-gate at 8/8).

All narrow softmax tiles (L, S1, R4) share one PSUM bank at base
partitions 0/32/64 (the legal matmul output bases).

Codegen constraint honored throughout: TRN2 instructions support a single
sync wait; Bacc's generate_event_semaphores pass splits the rest.
"""

import os
import sys
from contextlib import ExitStack

import numpy as np

try:
    import concourse.bass as bass
except ImportError:  # fresh grading dir: concourse lives in the container repo
    sys.path.insert(0, "/opt/trn_rl_repo")
    import concourse.bass as bass

import concourse.tile as tile
from concourse import bacc, mybir
from concourse.bass_utils import run_bass_kernel_spmd

B, C, H, W = 8, 256, 96, 96
HW = H * W  # 9216
NCORES = 8
CPB = C // 128  # 2 partition chunks per core
DCOLS = 1024  # columns per DMA tile
JCOLS = 512  # columns per compute slice (= fp32 PSUM bank)
NDMA = HW // DCOLS  # 9
NJ = DCOLS // JCOLS  # 2

F32 = mybir.dt.float32
F32R = mybir.dt.float32r
AF = mybir.ActivationFunctionType


def dve_takes(c, i):
    """Product (chunk c, grid i) on DVE? Rest go to gpsimd."""
    return i == 0 or (i == 1 and c == 0)


GPSIMD_GRIDS = (1, 2, 3)  # grids needing an SBUF-staged weight copy

_CACHE = {}


def build_program():
    nc = bacc.Bacc("TRN2", debug=False, num_devices=NCORES)

    gall_d = nc.dram_tensor("gall", [4, C, HW], F32R, kind="ExternalInput").ap()
    # one blob for all constants -> single DMA, single semaphore lane.
    # cols: 0-15 ws | 16 bvec | 17 ones4 | 18-529 selmat | 530-657 ident
    #       | 658-661 ones1x4
    cb_d = nc.dram_tensor("cblob", [128, 662], F32R, kind="ExternalInput").ap()
    out = nc.dram_tensor("out", [C, HW], F32, kind="ExternalOutput").ap()

    with tile.TileContext(nc) as tc, ExitStack() as ctx:
        const = ctx.enter_context(tc.tile_pool(name="const", bufs=1))
        gin = ctx.enter_context(tc.tile_pool(name="gin", bufs=2))
        outp = ctx.enter_context(tc.tile_pool(name="outp", bufs=2))
        narrow = ctx.enter_context(tc.tile_pool(name="narrow", bufs=3))
        wbsb = ctx.enter_context(tc.tile_pool(name="wbsb", bufs=2))
        prod = ctx.enter_context(tc.tile_pool(name="prod", bufs=20))
        ps_smx = ctx.enter_context(tc.tile_pool(name="psmx", bufs=2, space="PSUM"))
        ps_Wb = ctx.enter_context(tc.tile_pool(name="psWb", bufs=1, space="PSUM"))
        ps_F = ctx.enter_context(tc.tile_pool(name="psF", bufs=2, space="PSUM"))

        # constants -> SBUF in one DMA
        cb = const.tile([128, 662], F32R)
        nc.sync.dma_start(out=cb[:], in_=cb_d)
        ws = cb[:, 0:16]
        bv = cb[0:4, 16:17].bitcast(F32)
        ones4 = cb[0:4, 17:18]
        selmat = cb[0:4, 18:530]
        ident = cb[:, 530:658]
        ones1x4 = cb[0:1, 658:662]

        # Warmup matmul: absorbs the const-blob DMA wait on the PE clock.
        warm = ps_F.tile([4, 16], F32, tag="F")
        nc.tensor.matmul(warm[:], lhsT=ws[:, 0:4], rhs=ws, start=True, stop=True)

        def pass1(d, gat, state):
            """logits/softmax/broadcast/products for iter d."""
            for j in range(NJ):
                x0 = j * JCOLS
                smx = ps_smx.tile([128, JCOLS], F32)
                L = smx[0:4, :]
                # R4 reuses partitions 0-3 (L is dead after the exp reads it),
                # keeping every elementwise consumer partition-aligned.
                R4 = smx[0:4, :]
                # S1 borrows the wb0 bank before the Wb broadcasts need it;
                # it must sit at partition 0: the custom reciprocal DVE op
                # malfunctions at a nonzero base partition (HW-verified).
                S1 = ps_Wb.tile([1, JCOLS], F32, tag="wb0")

                k = 0
                for i in range(4):
                    for c in range(CPB):
                        nc.tensor.matmul(
                            L,
                            lhsT=ws[:, 4 * i : 4 * i + 4],
                            rhs=gat[:, i, c, x0 : x0 + JCOLS],
                            start=(k == 0),
                            stop=(k == 7),
                        )
                        k += 1
                E = narrow.tile([4, JCOLS], F32, tag="E")
                nc.scalar.activation(E[:], L, AF.Exp, bias=bv, scale=1.0)
                nc.tensor.matmul(
                    S1[:], lhsT=ones4.bitcast(F32), rhs=E[:], start=True, stop=True
                )
                R = narrow.tile([1, JCOLS], F32, tag="R")
                nc.vector.reciprocal_approx_fast(R[:], S1[:])
                nc.tensor.matmul(
                    R4, lhsT=ones1x4.bitcast(F32), rhs=R[:], start=True, stop=True
                )
                W4 = narrow.tile([4, JCOLS], F32R, tag="W4")
                nc.vector.tensor_mul(W4[:], E[:], R4.bitcast(F32))

                wbp = []
                wbs = {}
                for i in range(4):
                    Wbp = ps_Wb.tile([128, JCOLS], F32, tag=f"wb{i}")
                    nc.tensor.matmul(
                        Wbp[:],
                        lhsT=selmat[:, 128 * i : 128 * (i + 1)],
                        rhs=W4[:],
                        start=True,
                        stop=True,
                    )
                    wbp.append(Wbp)
                    if i in GPSIMD_GRIDS:
                        Wb = wbsb.tile([128, JCOLS], F32, tag=f"wbs{i}")
                        nc.scalar.copy(Wb[:], Wbp[:])
                        wbs[i] = Wb

                for c in range(CPB):
                    for i in range(4):
                        p = prod.tile([128, JCOLS], F32R, tag="p")
                        gslice = gat[:, i, c, x0 : x0 + JCOLS].bitcast(F32)
                        if dve_takes(c, i):
                            nc.vector.tensor_mul(p[:], gslice, wbp[i][:])
                        else:
                            nc.gpsimd.tensor_mul(p[:], gslice, wbs[i][:])
                        state[(j, c, i)] = p

        def pass2(d, ot, state):
            """PE accumulation + output copies + store for iter d."""
            for j in range(NJ):
                x0 = j * JCOLS
                for c in range(CPB):
                    F = ps_F.tile([128, JCOLS], F32, tag="F")
                    for i in range(4):
                        nc.tensor.matmul(
                            F[:],
                            lhsT=ident,
                            rhs=state[(j, c, i)][:],
                            start=(i == 0),
                            stop=(i == 3),
                        )
                    nc.vector.tensor_copy(ot[:, c, x0 : x0 + JCOLS], F[:])
            n0 = d * DCOLS
            nc.sync.dma_start(
                out=out[:, n0 : n0 + DCOLS].rearrange("(c p) n -> p c n", c=CPB),
                in_=ot[:],
            )

        prev = None
        for d in range(NDMA):
            n0 = d * DCOLS
            gat = gin.tile([128, 4, CPB, DCOLS], F32R, tag="gall")
            nc.sync.dma_start(
                out=gat[:],
                in_=gall_d[:, :, n0 : n0 + DCOLS].rearrange(
                    "i (c p) n -> p i c n", c=CPB
                ),
            )
            if prev is not None:
                pass2(*prev)
            ot = outp.tile([128, CPB, DCOLS], F32)
            state = {}
            pass1(d, gat, state)
            prev = (d, ot, state)
        pass2(*prev)

    nc.compile()
    return nc


def _get_program():
    if "nc" not in _CACHE:
        _CACHE["nc"] = build_program()
    return _CACHE["nc"]


LAST_RESULT = None


def kernel(g0, g1, g2, g3, w_proj, b_proj):
    global LAST_RESULT
    nc = _get_program()

    w = np.asarray(w_proj, dtype=np.float32)
    b = np.asarray(b_proj, dtype=np.float32)
    ws = np.empty((128, 16), dtype=np.float32)
    for i in range(4):
        for o in range(4):
            ws[:, 4 * i + o] = w[o, i] / C

    cblob = np.zeros((128, 662), dtype=np.float32)
    cblob[:, 0:16] = ws
    cblob[0:4, 16] = b
    cblob[0:4, 17] = 1.0
    cblob[0:4, 18:530] = np.repeat(np.eye(4, dtype=np.float32), 128, axis=1)
    cblob[:, 530:658] = np.eye(128, dtype=np.float32)
    cblob[0, 658:662] = 1.0

    gall = np.stack(
        [np.asarray(x, dtype=np.float32).reshape(B, C, HW) for x in (g0, g1, g2, g3)],
        axis=1,
    )  # (B, 4, C, HW)
    in_maps = []
    for bi in range(NCORES):
        m = {"gall": np.ascontiguousarray(gall[bi]), "cblob": cblob}
        in_maps.append(m)

    res = run_bass_kernel_spmd(
        nc,
        in_maps,
        list(range(NCORES)),
        trace=bool(int(os.environ.get("CM_TRACE", "0"))),
        tmpdir=os.environ.get("CM_TRACE_DIR") or None,
    )
    LAST_RESULT = res
    out_full = np.stack(
        [res.results[bi]["out"].reshape(C, H, W) for bi in range(NCORES)], axis=0
    )
    return out_full
